# revision 40
# baseline (speedup 1.0000x reference)
"""Trainium2 Bass kernel for nn_Decoder_24541443129406.

Math: the reference's pdf/pdf_max cancels the normalization, so

    prob[n] = clip( sum_m exp( -0.5 * sum_d (pos[n,d]-mean[m,d])^2 / sigma[m,d] ), 0, 1 )

with pos = [ox, oy, dx, dy], sigma = [sx, sy, 1e-3, 1e-3],
sx = relu(l4)+0.01, sy = relu(l5)+0.01, mean = latents[:, :4].

The exponent is a quadratic form -> a K=8 matmul per (ray, gaussian):
    e[n,m] = f[n] . w[m]
    f[n] = [dx^2+dy^2, 1, ox, oy, dx, dy, ox^2, oy^2]
    w[m] = [c7, c0, c1, c2, c3, c4, c5, c6]
      c1 = mx/sx, c2 = my/sy, c3 = 1000*mdx, c4 = 1000*mdy,
      c5 = -0.5/sx, c6 = -0.5/sy, c7 = -500,
      c0 = -0.5*(mx^2/sx + my^2/sy + 1000*(mdx^2+mdy^2))

fp32 matmuls are 4 cycles/row on the PE, so the K=8 fp32 matmul is
emulated in fp16 with hi/lo split operands stacked along K=24:
weight rows [H; H; L] pair with feature slots [h; lo; h], giving
e = h.H + lo.H + h.L (~2^-22 relative accuracy, 1 cycle/row).

Layout (per core, 8192 rays x 512 gaussians):
- rays p-major: partition p holds rays 64p..64p+63 (ray = 64p + r).
  origins+directions are host-interleaved into one rays [128, 256]
  array (col = 4r + {ox,oy,dx,dy}): a single contiguous input DMA.
- features: strided DVE writes into FT [128, 32*64 (+pad)] f16
  (col = 32r + 8q + f; q=0 h, q=1 lo, q=2 h-dup, q=3 unused), then one
  PE transpose [128,128] per ray-column g whose rows 0..23 are the
  block-g lhsT [24, 128]; 4 transposes land in one [32, 512] PSUM tile,
  copied to SBUF featT tiles (all matmuls at partition base 0).
- weights: ALU on [32, 16/32] views (DVE), hi/lo split into WALL
  (c7 = -500 rows are tiny memsets), then a DRAM bounce (SBUF APs cannot
  place the partition axis mid-AP, DRAM APs can) -> wgs [32, 512] f16.
- main loop: 22 units x (3 matmuls -> PSUM [128, 1536] f32 -> one ACT
  Exp -> fp16 SBUF -> 3 DVE tensor_scalar folds with accum_out ->
  res[:, block]).  ACT does only the Exps (~31us) = the bottleneck.
- res [128, 64] is already prob[64p + r]: two contiguous DMAs out
  (first half mid-loop to shorten the tail).  Folds lag their unit by 2
  so DVE transpose-copies are never queued behind exp-gated folds.
"""

import os
import sys
from contextlib import ExitStack

import numpy as np

for _p in ("/opt/trn_rl_repo", "/root/.axon_site/_ro/trn_rl_repo"):
    if os.path.isdir(_p) and _p not in sys.path:
        sys.path.insert(0, _p)

import concourse.bacc as bacc
import concourse.bass as bass
import concourse.mybir as mybir
import concourse.tile as tile
from concourse import bass_utils
from concourse.masks import make_identity

N_CORES = 8
N = 65536
M = 512
N_LOC = N // N_CORES  # 8192
RCOLS = N_LOC // 128  # 64 ray-columns; ray = 64p + r
NT = RCOLS // 4  # 16 featT tiles (4 ray-cols each)
BPU = 3  # blocks per main-loop unit ([128, 1536] PSUM tile)
NU = (RCOLS + BPU - 1) // BPU  # 22 units (last has 1 block)
TRB_AFTER_UNIT = 4  # emit second-half transposes after this unit
SIGMA_EPS = 0.01
INV_SDIR = 1000.0  # 1/sigma_dir

F32 = mybir.dt.float32
F16 = mybir.dt.float16
ALU = mybir.AluOpType
ACTF = mybir.ActivationFunctionType

TRACE = False
LAST_PERF = None
_CACHED_NC = None


def build_kernel_body(nc, rays, latents, prob, wtmp):
    """rays: [128, 256] f32 DRAM (col = 4r + {ox,oy,dx,dy});
    latents [32, 96]; prob [128, 64]."""
    with tile.TileContext(nc) as tc, ExitStack() as ctx:
        singles = ctx.enter_context(tc.tile_pool(name="singles", bufs=1))
        scratch = ctx.enter_context(tc.tile_pool(name="scratch", bufs=6))
        folds = ctx.enter_context(tc.tile_pool(name="folds", bufs=4))

        # ---------------- input loads ----------------
        # Both on SP/HWDGE: latents first (the weight path is the longest
        # dependency chain); rays lands ~1us later, clearing the DVE window
        # for the weight ALU before feature ops become ready.
        lat32 = singles.tile([32, 96], F32)
        nc.sync.dma_start(out=lat32, in_=latents)
        raw = singles.tile([128, 4 * RCOLS], F32)
        nc.sync.dma_start(out=raw, in_=rays)

        # f16 identity for the feature transposes
        ident = singles.tile([128, 128], F16)
        make_identity(nc, ident)

        # FT [128, 2048+96] f16, col = 32r + 8q + f (q=0 h, 1 lo, 2 h-dup,
        # 3 unused).  The 96 pad cols only feed transpose rows >= 24, which
        # no matmul reads.  The constant "ones" feature (f=1) has no input
        # deps: write it now, while the input DMAs fly.
        FT = singles.tile([128, 32 * RCOLS + 96], F16)
        FT4 = FT[:, 0 : 32 * RCOLS].rearrange("p (r q f) -> p r q f", q=4, f=8)
        nc.vector.memset(FT4[:, :, 0, 1], 1.0)
        nc.vector.memset(FT4[:, :, 1, 1], 0.0)
        nc.vector.memset(FT4[:, :, 2, 1], 1.0)
        # q=3 slots + tail pad are read by the transposes (rows >= 24 of the
        # transpose output, never consumed by matmuls) - zero them once
        nc.gpsimd.memset(FT4[:, :, 3, :], 0.0)
        nc.gpsimd.memset(FT[:, 32 * RCOLS :], 0.0)

        # ---------------- gaussian weights (DVE, [32, x] views) --------------
        latv = lat32.rearrange("p (j f) -> p j f", f=6)

        # P [32, 128] f32: piece r at cols 16r..16r+16, order
        # [c7, c0, c1, c2, c3, c4, c5, c6]; c7 written as a constant in WALL.
        P = singles.tile([32, 128], F32)

        def fmaj(apv):
            # [32, 32] slice -> (j, f) iteration of f-major storage
            return apv.rearrange("p (f j) -> p j f", f=2)

        sxy = singles.tile([32, 32], F32)
        sxy_v = sxy.rearrange("p (j f) -> p j f", f=2)
        nc.vector.tensor_scalar(
            out=sxy_v, in0=latv[:, :, 4:6], scalar1=0.0, scalar2=SIGMA_EPS,
            op0=ALU.max, op1=ALU.add,
        )
        rxy = singles.tile([32, 32], F32)
        rxy_v = rxy.rearrange("p (j f) -> p j f", f=2)
        nc.vector.reciprocal_approx_fast(out=rxy, in_=sxy)

        # c1, c2 = m_xy / s_xy
        nc.vector.tensor_tensor(
            out=fmaj(P[:, 32:64]), in0=latv[:, :, 0:2], in1=rxy_v, op=ALU.mult
        )
        # c3, c4 = 1000 * md_xy
        nc.vector.tensor_scalar_mul(
            out=fmaj(P[:, 64:96]), in0=latv[:, :, 2:4], scalar1=INV_SDIR
        )
        # c5, c6 = -0.5 / s_xy
        nc.vector.tensor_scalar_mul(
            out=fmaj(P[:, 96:128]), in0=rxy_v, scalar1=-0.5
        )
        # c0 = -0.5*(mx*c1 + my*c2) - 500*(mdx^2 + mdy^2)
        # q = (mx^2, my^2), qd = (mdx^2, mdy^2): both independent of recip
        q = singles.tile([32, 32], F32)
        qd = singles.tile([32, 32], F32)
        nc.vector.tensor_tensor(
            out=fmaj(q), in0=latv[:, :, 0:2], in1=latv[:, :, 0:2], op=ALU.mult
        )
        nc.vector.tensor_tensor(
            out=fmaj(qd), in0=latv[:, :, 2:4], in1=latv[:, :, 2:4], op=ALU.mult
        )
        vsum = singles.tile([32, 16], F32)
        nc.vector.tensor_add(out=vsum, in0=qd[:, 0:16], in1=qd[:, 16:32])
        nc.vector.tensor_scalar_mul(out=vsum, in0=vsum, scalar1=-0.5 * INV_SDIR)
        # qr = -0.5 * m^2 / s (after recip); c0 = qr_x + qr_y + vsum
        qr = singles.tile([32, 32], F32)
        nc.vector.scalar_tensor_tensor(
            out=fmaj(qr), in0=fmaj(q), scalar=-0.5, in1=rxy_v,
            op0=ALU.mult, op1=ALU.mult,
        )
        nc.vector.tensor_add(out=P[:, 16:32], in0=qr[:, 0:16], in1=qr[:, 16:32])
        nc.vector.tensor_add(out=P[:, 16:32], in0=P[:, 16:32], in1=vsum)

        # hi/lo split into WALL [32, 384] f16 = [H | H | L], c7 = const
        WALL = singles.tile([32, 384], F16)
        nc.vector.memset(WALL[:, 0:16], -0.5 * INV_SDIR)
        nc.vector.memset(WALL[:, 128:144], -0.5 * INV_SDIR)
        nc.vector.memset(WALL[:, 256:272], 0.0)
        nc.vector.tensor_copy(out=WALL[:, 16:128], in_=P[:, 16:128])
        nc.vector.tensor_copy(out=WALL[:, 144:256], in_=WALL[:, 16:128])
        nc.vector.tensor_tensor(
            out=WALL[:, 272:384], in0=P[:, 16:128], in1=WALL[:, 16:128],
            op=ALU.subtract,
        )

        # gather: SBUF APs cannot put the partition axis mid-AP, so bounce
        # WALL through DRAM (arbitrary strides are legal on the DRAM side):
        # wgs row (8g + r) <- wtmp[(p, 16r + j)] for m = 16p + j.
        wgs = singles.tile([32, M], F16)
        nc.sync.dma_start(out=wtmp, in_=WALL)
        wsrc = wtmp.rearrange("p (g r j) -> (g r) p j", g=3, j=16)
        nc.sync.dma_start(out=wgs[0:24, :], in_=wsrc)

        # ---------------- features (DVE, 2 column-chunks) --------------------
        ox, oy = raw[:, 0::4], raw[:, 1::4]
        dx, dy = raw[:, 2::4], raw[:, 3::4]

        t0 = singles.tile([128, RCOLS], F32)
        t1 = singles.tile([128, RCOLS], F32)
        t6 = singles.tile([128, RCOLS], F32)
        t7 = singles.tile([128, RCOLS], F32)

        def features(c):
            F4 = FT4[:, c]
            # f: 0=dx^2+dy^2, 1=1, 2=ox, 3=oy, 4=dx, 5=dy, 6=ox^2, 7=oy^2
            # h path (and squares) on DVE; lo subtracts + h-dup on gpsimd,
            # which is otherwise idle, so the DVE frees up for the weight ALU
            nc.vector.tensor_mul(out=t0[:, c], in0=dx[:, c], in1=dx[:, c])
            nc.vector.tensor_mul(out=t1[:, c], in0=dy[:, c], in1=dy[:, c])
            nc.vector.tensor_add(out=t0[:, c], in0=t0[:, c], in1=t1[:, c])
            nc.vector.tensor_copy(out=F4[:, :, 0, 0], in_=t0[:, c])
            nc.gpsimd.tensor_tensor(
                out=F4[:, :, 1, 0], in0=t0[:, c], in1=F4[:, :, 0, 0],
                op=ALU.subtract,
            )
            for f, view in ((2, ox), (3, oy), (4, dx), (5, dy)):
                nc.vector.tensor_copy(out=F4[:, :, 0, f], in_=view[:, c])
                nc.gpsimd.tensor_tensor(
                    out=F4[:, :, 1, f], in0=view[:, c], in1=F4[:, :, 0, f],
                    op=ALU.subtract,
                )
            nc.vector.tensor_mul(out=t6[:, c], in0=ox[:, c], in1=ox[:, c])
            nc.vector.tensor_copy(out=F4[:, :, 0, 6], in_=t6[:, c])
            nc.gpsimd.tensor_tensor(
                out=F4[:, :, 1, 6], in0=t6[:, c], in1=F4[:, :, 0, 6],
                op=ALU.subtract,
            )
            nc.vector.tensor_mul(out=t7[:, c], in0=oy[:, c], in1=oy[:, c])
            nc.vector.tensor_copy(out=F4[:, :, 0, 7], in_=t7[:, c])
            nc.gpsimd.tensor_tensor(
                out=F4[:, :, 1, 7], in0=t7[:, c], in1=F4[:, :, 0, 7],
                op=ALU.subtract,
            )
            # h dup: slots q=2 <- q=0 (f=1 already set by memset)
            nc.gpsimd.tensor_copy(out=F4[:, :, 2, 0], in_=F4[:, :, 0, 0])
            nc.gpsimd.tensor_copy(out=F4[:, :, 2, 2:8], in_=F4[:, :, 0, 2:8])

        # ---------------- main pipeline ----------------
        res = singles.tile([128, RCOLS], F32)  # res[p, r] = prob(ray 64p + r)
        featT = [
            singles.tile([32, 512], F16, name=f"featT{t}", tag=f"featT{t}")
            for t in range(NT)
        ]

        def transposes(trange, pstage):
            # one [128,128] transpose per ray-col g: rows 0..23 of the output
            # are block g's lhsT.  4 outputs share one [32, 512] PSUM tile.
            for t in trange:
                pst = pstage.tile([128, 512], F16, tag="pst")
                for k in range(4):
                    g = 4 * t + k
                    nc.tensor.transpose(
                        out=pst[:, 128 * k : 128 * (k + 1)],
                        in_=FT[:, 32 * g : 32 * g + 128],
                        identity=ident,
                    )
                nc.vector.tensor_copy(out=featT[t], in_=pst[0:32, :])

        pending_folds = []

        def do_folds():
            ex, blocks = pending_folds.pop(0)
            for i, b in enumerate(blocks):
                sc = folds.tile([128, 512], F16, tag="sc")
                nc.vector.tensor_scalar(
                    out=sc,
                    in0=ex[:, 512 * i : 512 * (i + 1)],
                    scalar1=0.0,
                    scalar2=0.0,
                    op0=ALU.add,
                    op1=ALU.add,
                    accum_out=res[:, b : b + 1],
                )

        def unit(u, psum):
            blocks = [b for b in range(BPU * u, min(BPU * (u + 1), RCOLS))]
            width = 512 * len(blocks)
            ps = psum.tile([128, 1536], F32, tag="ps")
            for i, b in enumerate(blocks):
                t, k = b // 4, b % 4
                nc.tensor.matmul(
                    out=ps[:, 512 * i : 512 * (i + 1)],
                    lhsT=featT[t][0:24, 128 * k : 128 * (k + 1)],
                    rhs=wgs[0:24, :],
                    start=True,
                    stop=True,
                )
            ex = scratch.tile([128, 1536], F16, tag="ex")
            nc.scalar.activation(out=ex[:, 0:width], in_=ps[:, 0:width], func=ACTF.Exp)
            # folds lag 2 units so DVE transpose-copies are never queued
            # behind exp-gated folds
            pending_folds.append((ex, blocks))
            if len(pending_folds) > 2:
                do_folds()

        with tc.tile_pool(name="pstage", bufs=2, space="PSUM") as pstage:
            with tc.tile_pool(name="psum", bufs=2, space="PSUM") as psum:
                features(slice(0, RCOLS // 2))
                # t0-t2 up front, then one transpose quad in each PE idle
                # window after a unit's matmuls; unit u needs featT up to
                # (3u+2)//4 <= u+2, which is emitted before unit u
                transposes(range(0, 3), pstage)
                unit(0, psum)
                transposes(range(3, 4), pstage)
                unit(1, psum)
                transposes(range(4, 5), pstage)
                features(slice(RCOLS // 2, RCOLS))
                for u in range(2, NU):
                    unit(u, psum)
                    if u + 3 < NT:
                        transposes(range(u + 3, u + 4), pstage)
                    if u == 12:
                        # folds lag 2 units: after unit 12, blocks 0..31 done;
                        # store the first half
                        nc.vector.tensor_scalar(
                            out=res[:, 0:32], in0=res[:, 0:32],
                            scalar1=0.0, scalar2=1.0, op0=ALU.max, op1=ALU.min,
                        )
                        nc.sync.dma_start(out=prob[:, 0:32], in_=res[:, 0:32])

        while pending_folds:
            do_folds()
        nc.vector.tensor_scalar(
            out=res[:, 32:], in0=res[:, 32:],
            scalar1=0.0, scalar2=1.0, op0=ALU.max, op1=ALU.min,
        )
        nc.sync.dma_start(out=prob[:, 32:], in_=res[:, 32:])


def build_nc():
    nc = bacc.Bacc("TRN2", target_bir_lowering=False, debug=False)
    rays = nc.dram_tensor("rays", [128, 4 * RCOLS], F32, kind="ExternalInput").ap()
    latents = nc.dram_tensor("latents", [32, 96], F32, kind="ExternalInput").ap()
    prob = nc.dram_tensor("prob", [128, RCOLS], F32, kind="ExternalOutput").ap()
    wtmp = nc.dram_tensor("wtmp", [32, 384], F16, kind="Internal").ap()
    build_kernel_body(nc, rays, latents, prob, wtmp)
    nc.compile()
    return nc


def kernel(origins: np.ndarray, directions: np.ndarray, latents: np.ndarray) -> np.ndarray:
    global _CACHED_NC, LAST_PERF
    assert origins.shape == (N, 2) and directions.shape == (N, 2)
    assert latents.shape == (M, 6)
    origins = np.ascontiguousarray(origins, dtype=np.float32)
    directions = np.ascontiguousarray(directions, dtype=np.float32)
    latents = np.ascontiguousarray(latents, dtype=np.float32)
    lat_r = latents.reshape(32, 96)

    if _CACHED_NC is None:
        _CACHED_NC = build_nc()
    nc = _CACHED_NC

    in_maps = []
    for c in range(N_CORES):
        sl = slice(c * N_LOC, (c + 1) * N_LOC)
        # interleave to [128, 64, 4] = (ox, oy, dx, dy) per ray, p-major
        rays = np.concatenate(
            [
                origins[sl].reshape(128, RCOLS, 2),
                directions[sl].reshape(128, RCOLS, 2),
            ],
            axis=2,
        ).reshape(128, 4 * RCOLS)
        in_maps.append(
            {
                "rays": np.ascontiguousarray(rays),
                "latents": lat_r,
            }
        )

    results = bass_utils.run_bass_kernel_spmd(
        nc,
        in_maps,
        core_ids=list(range(N_CORES)),
        trace=TRACE,
    )
    LAST_PERF = results
    out = np.concatenate(
        [results.results[c]["prob"].reshape(N_LOC, 1) for c in range(N_CORES)], axis=0
    )
    return out.astype(np.float32)


if __name__ == "__main__":
    rng = np.random.default_rng(0)
    o = rng.standard_normal((N, 2), dtype=np.float32)
    d = rng.standard_normal((N, 2), dtype=np.float32)
    l = rng.standard_normal((M, 6), dtype=np.float32)
    p = kernel(o, d, l)
    print(p.shape, p.dtype, p.min(), p.max())


# revision 42
# speedup vs baseline: 1.1879x; 1.1879x over previous
"""Trainium2 Bass kernel for nn_Decoder_24541443129406.

Math: the reference's pdf/pdf_max cancels the normalization, so

    prob[n] = clip( sum_m exp( -0.5 * sum_d (pos[n,d]-mean[m,d])^2 / sigma[m,d] ), 0, 1 )

with pos = [ox, oy, dx, dy], sigma = [sx, sy, 1e-3, 1e-3],
sx = relu(l4)+0.01, sy = relu(l5)+0.01, mean = latents[:, :4].

The exponent is a quadratic form -> a K=8 matmul per (ray, gaussian):
    e[n,m] = f[n] . w[m]
    f[n] = [dx^2+dy^2, 1, ox, oy, dx, dy, ox^2, oy^2]
    w[m] = [c7, c0, c1, c2, c3, c4, c5, c6]
      c1 = mx/sx, c2 = my/sy, c3 = 1000*mdx, c4 = 1000*mdy,
      c5 = -0.5/sx, c6 = -0.5/sy, c7 = -500,
      c0 = -0.5*(mx^2/sx + my^2/sy + 1000*(mdx^2+mdy^2))

fp32 matmuls are 4 cycles/row on the PE, so the K=8 fp32 matmul is
emulated in fp16 with hi/lo split operands stacked along K=24:
weight rows [H; H; L] pair with feature slots [h; lo; h], giving
e = h.H + lo.H + h.L (~2^-22 relative accuracy, 1 cycle/row).

Layout (per core, 8192 rays x 512 gaussians):
- rays p-major: partition p holds rays 64p..64p+63 (ray = 64p + r).
  origins+directions are host-interleaved into one rays [128, 256]
  array (col = 4r + {ox,oy,dx,dy}): a single contiguous input DMA.
- features: strided DVE writes into FT [128, 32*64 (+pad)] f16
  (col = 32r + 8q + f; q=0 h, q=1 lo, q=2 h-dup, q=3 unused), then one
  PE transpose [128,128] per ray-column g whose rows 0..23 are the
  block-g lhsT [24, 128]; 4 transposes land in one [32, 512] PSUM tile,
  copied to SBUF featT tiles (all matmuls at partition base 0).
- weights: ALU on [32, 16/32] views (DVE), hi/lo split into WALL
  (c7 = -500 rows are tiny memsets), then a DRAM bounce (SBUF APs cannot
  place the partition axis mid-AP, DRAM APs can) -> wgs [32, 512] f16.
- main loop: 22 units x (3 matmuls -> PSUM [128, 1536] f32 -> one ACT
  Exp -> fp16 SBUF -> 3 DVE tensor_scalar folds with accum_out ->
  res[:, block]).  ACT does only the Exps (~31us) = the bottleneck.
- res [128, 64] is already prob[64p + r]: two contiguous DMAs out
  (first half mid-loop to shorten the tail).  Folds lag their unit by 2
  so DVE transpose-copies are never queued behind exp-gated folds.
"""

import os
import sys
from contextlib import ExitStack

import numpy as np

for _p in ("/opt/trn_rl_repo", "/root/.axon_site/_ro/trn_rl_repo"):
    if os.path.isdir(_p) and _p not in sys.path:
        sys.path.insert(0, _p)

import concourse.bacc as bacc
import concourse.bass as bass
import concourse.mybir as mybir
import concourse.tile as tile
from concourse import bass_utils
from concourse.masks import make_identity

N_CORES = 8
N = 65536
M = 512
N_LOC = N // N_CORES  # 8192
RCOLS = N_LOC // 128  # 64 ray-columns; ray = 64p + r
NT = RCOLS // 4  # 16 featT tiles (4 ray-cols each)
BPU = 3  # blocks per main-loop unit ([128, 1536] PSUM tile)
NU = (RCOLS + BPU - 1) // BPU  # 22 units (last has 1 block)
TRB_AFTER_UNIT = 4  # emit second-half transposes after this unit
SIGMA_EPS = 0.01
INV_SDIR = 1000.0  # 1/sigma_dir

F32 = mybir.dt.float32
F16 = mybir.dt.float16
ALU = mybir.AluOpType
ACTF = mybir.ActivationFunctionType

TRACE = False
LAST_PERF = None
_CACHED_NC = None


def build_kernel_body(nc, rays, latents, prob, wtmp):
    """rays: [128, 256] f32 DRAM (col = 4r + {ox,oy,dx,dy});
    latents [32, 96]; prob [128, 64]."""
    with tile.TileContext(nc) as tc, ExitStack() as ctx:
        singles = ctx.enter_context(tc.tile_pool(name="singles", bufs=1))
        scratch = ctx.enter_context(tc.tile_pool(name="scratch", bufs=6))
        folds = ctx.enter_context(tc.tile_pool(name="folds", bufs=4))

        # ---------------- input loads ----------------
        # Both on SP/HWDGE: latents first (the weight path is the longest
        # dependency chain); rays lands ~1us later, clearing the DVE window
        # for the weight ALU before feature ops become ready.
        lat32 = singles.tile([32, 96], F32)
        nc.sync.dma_start(out=lat32, in_=latents)
        raw = singles.tile([128, 4 * RCOLS], F32)
        nc.sync.dma_start(out=raw, in_=rays)

        # f16 identity for the feature transposes
        ident = singles.tile([128, 128], F16)
        make_identity(nc, ident)

        # FT [128, 2048+96] f16, col = 32r + 8q + f (q=0 h, 1 lo, 2 h-dup,
        # 3 unused).  The 96 pad cols only feed transpose rows >= 24, which
        # no matmul reads.  The constant "ones" feature (f=1) has no input
        # deps: write it now, while the input DMAs fly.
        FT = singles.tile([128, 32 * RCOLS + 96], F16)
        FT4 = FT[:, 0 : 32 * RCOLS].rearrange("p (r q f) -> p r q f", q=4, f=8)
        nc.vector.memset(FT4[:, :, 0, 1], 1.0)
        nc.vector.memset(FT4[:, :, 1, 1], 0.0)
        nc.vector.memset(FT4[:, :, 2, 1], 1.0)
        # q=3 slots + tail pad are read by the transposes (rows >= 24 of the
        # transpose output, never consumed by matmuls) - zero them once
        nc.gpsimd.memset(FT4[:, :, 3, :], 0.0)
        nc.gpsimd.memset(FT[:, 32 * RCOLS :], 0.0)

        # ---------------- gaussian weights (DVE, [32, x] views) --------------
        latv = lat32.rearrange("p (j f) -> p j f", f=6)

        # P [32, 128] f32: piece r at cols 16r..16r+16, order
        # [c7, c0, c1, c2, c3, c4, c5, c6]; c7 written as a constant in WALL.
        P = singles.tile([32, 128], F32)

        def fmaj(apv):
            # [32, 32] slice -> (j, f) iteration of f-major storage
            return apv.rearrange("p (f j) -> p j f", f=2)

        sxy = singles.tile([32, 32], F32)
        sxy_v = sxy.rearrange("p (j f) -> p j f", f=2)
        nc.vector.tensor_scalar(
            out=sxy_v, in0=latv[:, :, 4:6], scalar1=0.0, scalar2=SIGMA_EPS,
            op0=ALU.max, op1=ALU.add,
        )
        rxy = singles.tile([32, 32], F32)
        rxy_v = rxy.rearrange("p (j f) -> p j f", f=2)
        nc.vector.reciprocal_approx_fast(out=rxy, in_=sxy)

        # c1, c2 = m_xy / s_xy
        nc.vector.tensor_tensor(
            out=fmaj(P[:, 32:64]), in0=latv[:, :, 0:2], in1=rxy_v, op=ALU.mult
        )
        # c3, c4 = 1000 * md_xy
        nc.vector.tensor_scalar_mul(
            out=fmaj(P[:, 64:96]), in0=latv[:, :, 2:4], scalar1=INV_SDIR
        )
        # c5, c6 = -0.5 / s_xy
        nc.vector.tensor_scalar_mul(
            out=fmaj(P[:, 96:128]), in0=rxy_v, scalar1=-0.5
        )
        # c0 = -0.5*(mx*c1 + my*c2) - 500*(mdx^2 + mdy^2)
        # q = (mx^2, my^2), qd = (mdx^2, mdy^2): both independent of recip
        q = singles.tile([32, 32], F32)
        qd = singles.tile([32, 32], F32)
        nc.vector.tensor_tensor(
            out=fmaj(q), in0=latv[:, :, 0:2], in1=latv[:, :, 0:2], op=ALU.mult
        )
        nc.vector.tensor_tensor(
            out=fmaj(qd), in0=latv[:, :, 2:4], in1=latv[:, :, 2:4], op=ALU.mult
        )
        vsum = singles.tile([32, 16], F32)
        nc.vector.tensor_add(out=vsum, in0=qd[:, 0:16], in1=qd[:, 16:32])
        nc.vector.tensor_scalar_mul(out=vsum, in0=vsum, scalar1=-0.5 * INV_SDIR)
        # qr = -0.5 * m^2 / s (after recip); c0 = qr_x + qr_y + vsum
        qr = singles.tile([32, 32], F32)
        nc.vector.scalar_tensor_tensor(
            out=fmaj(qr), in0=fmaj(q), scalar=-0.5, in1=rxy_v,
            op0=ALU.mult, op1=ALU.mult,
        )
        nc.vector.tensor_add(out=P[:, 16:32], in0=qr[:, 0:16], in1=qr[:, 16:32])
        nc.vector.tensor_add(out=P[:, 16:32], in0=P[:, 16:32], in1=vsum)

        # hi/lo split into WALL [32, 384] f16 = [H | H | L], c7 = const
        WALL = singles.tile([32, 384], F16)
        nc.vector.memset(WALL[:, 0:16], -0.5 * INV_SDIR)
        nc.vector.memset(WALL[:, 128:144], -0.5 * INV_SDIR)
        nc.vector.memset(WALL[:, 256:272], 0.0)
        nc.vector.tensor_copy(out=WALL[:, 16:128], in_=P[:, 16:128])
        nc.vector.tensor_copy(out=WALL[:, 144:256], in_=WALL[:, 16:128])
        nc.vector.tensor_tensor(
            out=WALL[:, 272:384], in0=P[:, 16:128], in1=WALL[:, 16:128],
            op=ALU.subtract,
        )

        # gather: SBUF APs cannot put the partition axis mid-AP, so bounce
        # WALL through DRAM (arbitrary strides are legal on the DRAM side):
        # wgs row (8g + r) <- wtmp[(p, 16r + j)] for m = 16p + j.
        wgs = singles.tile([32, M], F16)
        nc.sync.dma_start(out=wtmp, in_=WALL)
        wsrc = wtmp.rearrange("p (g r j) -> (g r) p j", g=3, j=16)
        nc.sync.dma_start(out=wgs[0:24, :], in_=wsrc)

        # ---------------- features (DVE, 2 column-chunks) --------------------
        ox, oy = raw[:, 0::4], raw[:, 1::4]
        dx, dy = raw[:, 2::4], raw[:, 3::4]

        t0 = singles.tile([128, RCOLS], F32)
        t1 = singles.tile([128, RCOLS], F32)
        t6 = singles.tile([128, RCOLS], F32)
        t7 = singles.tile([128, RCOLS], F32)

        def features(c):
            F4 = FT4[:, c]
            # f: 0=dx^2+dy^2, 1=1, 2=ox, 3=oy, 4=dx, 5=dy, 6=ox^2, 7=oy^2
            # h path (and squares) on DVE; lo subtracts + h-dup on gpsimd,
            # which is otherwise idle, so the DVE frees up for the weight ALU
            nc.vector.tensor_mul(out=t0[:, c], in0=dx[:, c], in1=dx[:, c])
            nc.vector.tensor_mul(out=t1[:, c], in0=dy[:, c], in1=dy[:, c])
            nc.vector.tensor_add(out=t0[:, c], in0=t0[:, c], in1=t1[:, c])
            nc.vector.tensor_copy(out=F4[:, :, 0, 0], in_=t0[:, c])
            nc.gpsimd.tensor_tensor(
                out=F4[:, :, 1, 0], in0=t0[:, c], in1=F4[:, :, 0, 0],
                op=ALU.subtract,
            )
            for f, view in ((2, ox), (3, oy), (4, dx), (5, dy)):
                nc.vector.tensor_copy(out=F4[:, :, 0, f], in_=view[:, c])
                nc.gpsimd.tensor_tensor(
                    out=F4[:, :, 1, f], in0=view[:, c], in1=F4[:, :, 0, f],
                    op=ALU.subtract,
                )
            nc.vector.tensor_mul(out=t6[:, c], in0=ox[:, c], in1=ox[:, c])
            nc.vector.tensor_copy(out=F4[:, :, 0, 6], in_=t6[:, c])
            nc.gpsimd.tensor_tensor(
                out=F4[:, :, 1, 6], in0=t6[:, c], in1=F4[:, :, 0, 6],
                op=ALU.subtract,
            )
            nc.vector.tensor_mul(out=t7[:, c], in0=oy[:, c], in1=oy[:, c])
            nc.vector.tensor_copy(out=F4[:, :, 0, 7], in_=t7[:, c])
            nc.gpsimd.tensor_tensor(
                out=F4[:, :, 1, 7], in0=t7[:, c], in1=F4[:, :, 0, 7],
                op=ALU.subtract,
            )
            # h dup: slots q=2 <- q=0 (f=1 already set by memset)
            nc.gpsimd.tensor_copy(out=F4[:, :, 2, 0], in_=F4[:, :, 0, 0])
            nc.gpsimd.tensor_copy(out=F4[:, :, 2, 2:8], in_=F4[:, :, 0, 2:8])

        # ---------------- main pipeline ----------------
        res = singles.tile([128, RCOLS], F32)  # res[p, r] = prob(ray 64p + r)
        featT = [
            singles.tile([32, 512], F16, name=f"featT{t}", tag=f"featT{t}")
            for t in range(NT)
        ]

        def transposes(trange, pstage):
            # one [128,128] transpose per ray-col g: rows 0..23 of the output
            # are block g's lhsT.  4 outputs share one [32, 512] PSUM tile.
            for t in trange:
                pst = pstage.tile([128, 512], F16, tag="pst")
                for k in range(4):
                    g = 4 * t + k
                    nc.tensor.transpose(
                        out=pst[:, 128 * k : 128 * (k + 1)],
                        in_=FT[:, 32 * g : 32 * g + 128],
                        identity=ident,
                    )
                nc.vector.tensor_copy(out=featT[t], in_=pst[0:32, :])

        pending_folds = []

        def do_folds():
            ex, blocks = pending_folds.pop(0)
            for i, b in enumerate(blocks):
                sc = folds.tile([128, 512], F16, tag="sc")
                nc.vector.tensor_scalar(
                    out=sc,
                    in0=ex[:, 512 * i : 512 * (i + 1)],
                    scalar1=0.0,
                    scalar2=0.0,
                    op0=ALU.add,
                    op1=ALU.add,
                    accum_out=res[:, b : b + 1],
                )

        def unit(u, psum):
            blocks = [b for b in range(BPU * u, min(BPU * (u + 1), RCOLS))]
            width = 512 * len(blocks)
            ps = psum.tile([128, 1536], F32, tag="ps")
            for i, b in enumerate(blocks):
                t, k = b // 4, b % 4
                nc.tensor.matmul(
                    out=ps[:, 512 * i : 512 * (i + 1)],
                    lhsT=featT[t][0:24, 128 * k : 128 * (k + 1)],
                    rhs=wgs[0:24, :],
                    start=True,
                    stop=True,
                )
            ex = scratch.tile([128, 1536], F16, tag="ex")
            nc.scalar.activation(out=ex[:, 0:width], in_=ps[:, 0:width], func=ACTF.Exp)
            # folds lag 2 units so DVE transpose-copies are never queued
            # behind exp-gated folds
            pending_folds.append((ex, blocks))
            if len(pending_folds) > 2:
                do_folds()

        with tc.tile_pool(name="pstage", bufs=2, space="PSUM") as pstage:
            with tc.tile_pool(name="psum", bufs=2, space="PSUM") as psum:
                features(slice(0, RCOLS // 2))
                # t0-t2 up front, then one transpose quad in each PE idle
                # window after a unit's matmuls; unit u needs featT up to
                # (3u+2)//4 <= u+2, which is emitted before unit u
                transposes(range(0, 3), pstage)
                unit(0, psum)
                transposes(range(3, 4), pstage)
                unit(1, psum)
                transposes(range(4, 5), pstage)
                features(slice(RCOLS // 2, RCOLS))
                for u in range(2, NU):
                    unit(u, psum)
                    if u + 3 < NT:
                        transposes(range(u + 3, u + 4), pstage)
                    if u == 12:
                        # folds lag 2 units: after unit 12, blocks 0..31 done;
                        # store the first half
                        nc.vector.tensor_scalar(
                            out=res[:, 0:32], in0=res[:, 0:32],
                            scalar1=0.0, scalar2=1.0, op0=ALU.max, op1=ALU.min,
                        )
                        nc.sync.dma_start(out=prob[:, 0:32], in_=res[:, 0:32])

        while pending_folds:
            do_folds()
        nc.vector.tensor_scalar(
            out=res[:, 32:], in0=res[:, 32:],
            scalar1=0.0, scalar2=1.0, op0=ALU.max, op1=ALU.min,
        )
        nc.sync.dma_start(out=prob[:, 32:], in_=res[:, 32:])


def build_nc():
    nc = bacc.Bacc("TRN2", target_bir_lowering=False, debug=False)
    rays = nc.dram_tensor("rays", [128, 4 * RCOLS], F32, kind="ExternalInput").ap()
    latents = nc.dram_tensor("latents", [32, 96], F32, kind="ExternalInput").ap()
    prob = nc.dram_tensor("prob", [128, RCOLS], F32, kind="ExternalOutput").ap()
    wtmp = nc.dram_tensor("wtmp", [32, 384], F16, kind="Internal").ap()
    build_kernel_body(nc, rays, latents, prob, wtmp)
    nc.compile()
    return nc


def kernel(origins: np.ndarray, directions: np.ndarray, latents: np.ndarray) -> np.ndarray:
    global _CACHED_NC, LAST_PERF
    assert origins.shape == (N, 2) and directions.shape == (N, 2)
    assert latents.shape == (M, 6)
    origins = np.ascontiguousarray(origins, dtype=np.float32)
    directions = np.ascontiguousarray(directions, dtype=np.float32)
    latents = np.ascontiguousarray(latents, dtype=np.float32)
    lat_r = latents.reshape(32, 96)

    if _CACHED_NC is None:
        _CACHED_NC = build_nc()
    nc = _CACHED_NC

    in_maps = []
    for c in range(N_CORES):
        sl = slice(c * N_LOC, (c + 1) * N_LOC)
        # interleave to [128, 64, 4] = (ox, oy, dx, dy) per ray, p-major
        rays = np.concatenate(
            [
                origins[sl].reshape(128, RCOLS, 2),
                directions[sl].reshape(128, RCOLS, 2),
            ],
            axis=2,
        ).reshape(128, 4 * RCOLS)
        in_maps.append(
            {
                "rays": np.ascontiguousarray(rays),
                "latents": lat_r,
            }
        )

    results = bass_utils.run_bass_kernel_spmd(
        nc,
        in_maps,
        core_ids=list(range(N_CORES)),
        trace=TRACE,
    )
    LAST_PERF = results
    out = np.concatenate(
        [results.results[c]["prob"].reshape(N_LOC, 1) for c in range(N_CORES)], axis=0
    )
    return out.astype(np.float32)


if __name__ == "__main__":
    rng = np.random.default_rng(0)
    o = rng.standard_normal((N, 2), dtype=np.float32)
    d = rng.standard_normal((N, 2), dtype=np.float32)
    l = rng.standard_normal((M, 6), dtype=np.float32)
    p = kernel(o, d, l)
    print(p.shape, p.dtype, p.min(), p.max())
